# revision 25
# baseline (speedup 1.0000x reference)
"""Multi-head attention forward (b=8, n=2048, dim=512, heads=8, dh=64) on 8
Trainium2 NeuronCores.

Sharding: data-parallel over batch — core i computes the full attention layer
for batch element i (weights replicated, no collectives).

Per-core pipeline (everything "transposed" so softmax rowsums come out of the
same matmuls):
  1. x [2048,512] -> PE-transpose -> xT [512,2048]
  2. qkT = w_qk.T @ xT           [1024,2048]  (q/k features on partitions;
     the q half is pre-scaled at weight-staging time)
  3. v   = x @ w_v               [2048,512]   (tokens on partitions) + ones col
  4. per head h, per pair of 512-wide query blocks:
       simT[j,i]  = kT_h[:,j].T @ qT_h[:,i]      (K=64, bf16 operands)
       exp        = Act engine Exp -> bf16       (256 x [128,1024] ops)
       outT_aug   = sum_j v_aug[j].T @ exp       (row 64 = exp rowsum)
       rinv       = 1/rowsum (DVE); bcast + multiply on gpsimd
       attn_outT[hd, i] = outT_aug[0:64] * rinv_bcast
  5. out = attn_outT.T @ w_out   [2048,512]

Tuned on hardware (defaults reflect the fastest measured config):
  - qkT/v/exp/attn_out/w_out in bf16: halves PE stationary-load (LDWEIGHTS)
    bytes in the attention inner loop (-25% wall). xT/w_qkv stay f32r (the
    all-bf16 projection variant miscomputed and was slower anyway).
  - attnv lags the sim matmul by 3 iterations (pss bufs=3, pso bufs=1, early
    PSUM->SBUF evacuation) so each exp has a ~2.5us window off the PE's
    critical path (-15% wall vs lag-1).
  - every PSUM->SBUF copy lives on the Act engine; the normalize multiply
    lives on gpsimd. The DVE is kept nearly empty: on this silicon its
    ops cost ~2x the cost model (mandatory inter-op DRAIN), and every
    attempt to offload exp (Schraudolph bit-trick) or copies onto it
    measured slower on HW despite the model predicting wins.

Env knobs (F_*) remain for hardware bisection; defaults are the tuned config.
"""

import os

import numpy as np

import concourse.bass as bass
import concourse.mybir as mybir
import concourse.tile as tile
from concourse import bacc
from concourse.masks import make_identity

FP32 = mybir.dt.float32
F32R = mybir.dt.float32r
BF16 = mybir.dt.bfloat16
I16 = mybir.dt.int16

B = 8
N = 2048
D = 512
H = 8
DH = 64
F3 = 3 * D
SCALE = DH**-0.5
P = 128
NT = N // P  # 16 token tiles
CT = D // P  # 4 contraction tiles over dim
JT = N // P  # 16 key tiles

# Schraudolph exp-in-bf16-bits: for w = e^s, bf16 bits ~= A16*s + 16256 - C
A16 = 184.6649652337873  # 128 * log2(e)
C_SCH = 5.5  # minimax-tuned bit offset
SCH_BIAS = 16256.0 - C_SCH + 0.5  # +0.5 so a truncating f32->i16 cast rounds
INV_A16 = 1.0 / A16

F_BF16 = os.environ.get("F_BF16", "1") == "1"
F_ACTCP = os.environ.get("F_ACTCP", "1") == "1"
F_LAG2 = os.environ.get("F_LAG2", "1") == "1"
# number of j-tiles (of 16) whose exp runs on the DVE (Schraudolph bits)
F_NDVE = int(os.environ.get("F_NDVE", "0"))
# normalize: 1 = recip on DVE + multiply on gpsimd (broadcast AP), 0 = old
# DVE recip + gpsimd partition_broadcast + DVE multiply
F_POOLMUL = os.environ.get("F_POOLMUL", "1") == "1"
# v_aug PSUM->SBUF copies: 1 = Act engine, 0 = DVE
F_VACT = os.environ.get("F_VACT", "1") == "1"
# pso evacuation engine: "act" or "dve"
F_EVAC = os.environ.get("F_EVAC", "act")
# timing-only knob: process only the first F_HLIM heads (wrong numerics)
F_HLIM = int(os.environ.get("F_HLIM", str(H)))
# sub-knobs: bf16 for qkT (sim operands) / xT+wqkv (proj operands)
F_QKBF = os.environ.get("F_QKBF", "1") == "1"
F_XWBF = os.environ.get("F_XWBF", "0") == "1"
# normalize divide on gpsimd (no DVE recip): 1 = bcast rowsum + divide
F_DIV = os.environ.get("F_DIV", "0") == "1"
# p2-outer loop order with early out-projection per half
F_P2OUT = os.environ.get("F_P2OUT", "0") == "1"
# cross-block software pipeline: defer each block's attnv tail until after
# the next block's first two sim matmuls (needs pss bufs=2 / pso bufs=2)
F_XBLK = os.environ.get("F_XBLK", "0") == "1"

F_DVE = F_NDVE > 0
MM_DT = BF16 if F_BF16 else F32R
QSCALE = (A16 * SCALE) if F_DVE else SCALE
EXP_SCALE = INV_A16 if F_DVE else 1.0
# j-tiles whose exp runs on the DVE instead of the Act engine, spread out
DVE_J = frozenset(
    (i * JT // F_NDVE + JT // (2 * F_NDVE)) for i in range(F_NDVE)
) if F_DVE else frozenset()


def _attention_body(tc: "tile.TileContext", repeat: int = 1):
    nc = tc.nc
    x = nc.dram_tensor("x", [N, D], FP32, kind="ExternalInput").ap()
    w_qkv = nc.dram_tensor("w_qkv", [D, F3], FP32, kind="ExternalInput").ap()
    w_out = nc.dram_tensor("w_out", [D, D], FP32, kind="ExternalInput").ap()
    out = nc.dram_tensor("out", [N, D], FP32, kind="ExternalOutput").ap()
    for _ in range(repeat):
        _attention_once(tc, x, w_qkv, w_out, out)


def _attention_once(tc: "tile.TileContext", x, w_qkv, w_out, out):
    nc = tc.nc
    exp_f = mybir.ActivationFunctionType.Exp

    with (
        tc.tile_pool(name="const", bufs=1) as const,
        tc.tile_pool(name="persist", bufs=1) as persist,
        tc.tile_pool(name="wstage", bufs=1) as wstage,
    ):
        identity = const.tile([P, P], FP32)
        make_identity(nc, identity)
        ones32 = const.tile([P, 1], FP32)
        nc.vector.memset(ones32, 1.0)

        # w_out: DMA fp32 -> rounding copy
        wout_sb = persist.tile([P, CT, D], MM_DT)
        with nc.allow_low_precision("weights"):
            for t in range(CT):
                ws = wstage.tile([P, F3], FP32, tag="ws")
                nc.sync.dma_start(out=ws[:, :D], in_=w_out[t * P : (t + 1) * P, :])
                nc.vector.tensor_copy(out=wout_sb[:, t, :], in_=ws[:, :D])

        # q and k features transposed: rows = 1024 q/k features in 8 tiles
        qkT = persist.tile([P, 8, N], MM_DT if F_QKBF else F32R)
        # v with tokens on partitions; per head 64 value cols + 1 ones col
        v_aug = persist.tile([P, JT, H * 65], MM_DT)
        with nc.allow_low_precision("v"):
            nc.vector.tensor_copy(
                out=v_aug.rearrange("p j (h c) -> p j h c", c=65)[:, :, :, 64:65],
                in_=ones32.to_broadcast([P, JT, H, 1]),
            )

        with (
            tc.tile_pool(name="proj", bufs=1) as proj_pool,
            tc.tile_pool(name="xstage", bufs=3) as xstage,
            tc.tile_pool(name="pst", bufs=3, space="PSUM") as pst,
            tc.tile_pool(name="psmm", bufs=2, space="PSUM") as psmm,
        ):
            # w_qkv staging; the q columns (0:512) are pre-scaled
            wqkv_sb = proj_pool.tile([P, CT, F3], MM_DT if F_XWBF else F32R)
            for t in range(CT):
                ws = wstage.tile([P, F3], FP32, tag="ws")
                nc.sync.dma_start(out=ws, in_=w_qkv[t * P : (t + 1) * P, :])
                nc.vector.tensor_scalar_mul(
                    out=wqkv_sb[:, t, 0:D], in0=ws[:, 0:D], scalar1=QSCALE
                )
                nc.vector.tensor_copy(out=wqkv_sb[:, t, D:F3], in_=ws[:, D:F3])

            # ---- load x and transpose to xT [512, 2048] ----
            xT = proj_pool.tile([P, CT, N], MM_DT if F_XWBF else F32R)
            for j in range(NT):
                xs = xstage.tile([P, D], FP32)
                nc.sync.dma_start(out=xs, in_=x[j * P : (j + 1) * P, :])
                ps = pst.tile([P, CT, P], FP32)  # one bank, 4 transposes
                for t in range(CT):
                    nc.tensor.transpose(
                        ps[:, t, :], xs[:, t * P : (t + 1) * P], identity
                    )
                nc.vector.tensor_copy(out=xT[:, :, j * P : (j + 1) * P], in_=ps)

            # ---- v = x @ w_v (tokens on partitions) ----
            for jp in range(8):
                ps = psmm.tile([P, 2, 512], FP32, tag="mm")
                for c in range(CT):
                    for q in range(2):
                        j = jp * 2 + q
                        nc.tensor.matmul(
                            ps[:, q, :],
                            xT[:, c, j * P : (j + 1) * P],
                            wqkv_sb[:, c, 2 * D : 3 * D],
                            start=(c == 0),
                            stop=(c == CT - 1),
                        )
                with nc.allow_low_precision("v"):
                    v_dst = v_aug[:, jp * 2 : jp * 2 + 2, :].rearrange(
                        "p j (h c) -> p j h c", c=65
                    )[:, :, :, 0:64]
                    v_src = ps.rearrange("p q (h c) -> p q h c", c=64)
                    if F_VACT:
                        nc.scalar.copy(out=v_dst, in_=v_src)
                    else:
                        nc.vector.tensor_copy(out=v_dst, in_=v_src)

            # ---- qkT = w_qk.T @ xT ----
            # m-order pairs each head's q tile with its k tile so head 0's
            # attention can start as early as possible
            for m in (0, 4, 1, 5, 2, 6, 3, 7):
                for nbp in range(2):
                    ps = psmm.tile([P, 2, 512], FP32, tag="mm")
                    for c in range(CT):
                        for q in range(2):
                            nc.tensor.matmul(
                                ps[:, q, :],
                                wqkv_sb[:, c, m * P : (m + 1) * P],
                                xT[:, c, (nbp * 2 + q) * 512 : (nbp * 2 + q + 1) * 512],
                                start=(c == 0),
                                stop=(c == CT - 1),
                            )
                    if F_ACTCP:
                        nc.scalar.copy(
                            out=qkT[:, m, nbp * 1024 : (nbp + 1) * 1024],
                            in_=ps.rearrange("p a b -> p (a b)"),
                        )
                    else:
                        nc.vector.tensor_copy(
                            out=qkT[:, m, nbp * 1024 : (nbp + 1) * 1024],
                            in_=ps.rearrange("p a b -> p (a b)"),
                        )

        # ---- attention per head, i-blocks in pairs ----
        with (
            tc.tile_pool(name="attno", bufs=1) as attno_pool,
            tc.tile_pool(name="expp", bufs=int(os.environ.get("F_EXPB", "5")) if F_LAG2 else 3) as expp,
            tc.tile_pool(name="rinvp", bufs=2) as rinvp,
            tc.tile_pool(name="psosb", bufs=2) as psosbp,
            tc.tile_pool(name="outstage", bufs=2) as outstage,
            tc.tile_pool(name="pss", bufs=3 if F_LAG2 else 2, space="PSUM") as pssp,
            tc.tile_pool(name="pso", bufs=1 if F_LAG2 else 2, space="PSUM") as psop,
        ):
            attn_outT = attno_pool.tile([P, CT, N], MM_DT)
            if F_HLIM < H:
                with nc.allow_low_precision("timing stub"):
                    nc.vector.memset(attn_outT, 0.25)

            def out_proj(jp):
                ps = pssp.tile([P, 2, 512], FP32, tag="mm")
                for t in range(CT):
                    for q in range(2):
                        j = jp * 2 + q
                        nc.tensor.matmul(
                            ps[:, q, :],
                            attn_outT[:, t, j * P : (j + 1) * P],
                            wout_sb[:, t, :],
                            start=(t == 0),
                            stop=(t == CT - 1),
                        )
                os_ = outstage.tile([P, 2, D], FP32)
                if F_ACTCP:
                    nc.scalar.copy(
                        out=os_.rearrange("p a b -> p (a b)"),
                        in_=ps.rearrange("p a b -> p (a b)"),
                    )
                else:
                    nc.vector.tensor_copy(out=os_, in_=ps)
                nc.sync.dma_start(
                    out=out[jp * 256 : (jp + 1) * 256, :].rearrange(
                        "(q p) d -> p q d", p=P
                    ),
                    in_=os_,
                )

            def make_tail(pending, attnv, pso, h, p2, qt, qo):
                # flush the last two attnv matmuls, evacuate PSUM, normalize,
                # and (after the final head's half) start that half's
                # out-projection
                def tail():
                    for pex, pj in pending:
                        attnv(pex, pj, stop=(pj == JT - 1))
                    pso_sb = psosbp.tile([65, 2, 512], FP32, tag="ps")
                    nc.scalar.copy(
                        out=pso_sb.rearrange("p a b -> p (a b)"),
                        in_=pso[0:65, :, :].rearrange("p a b -> p (a b)"),
                    )
                    rinv = rinvp.tile([1, 2, 512], FP32, tag="rinv")
                    nc.vector.reciprocal(out=rinv, in_=pso_sb[64:65, :, :])
                    rb = rinvp.tile([64, 2, 512], FP32, tag="rb")
                    nc.gpsimd.partition_broadcast(rb, rinv)
                    with nc.allow_low_precision("attn out"):
                        nc.gpsimd.tensor_mul(
                            out=attn_outT[
                                qo : qo + 64, qt, p2 * 1024 : (p2 + 1) * 1024
                            ].rearrange("p (a b) -> p a b", a=2),
                            in0=pso_sb[0:64, :, :],
                            in1=rb,
                        )
                    if h == F_HLIM - 1:
                        for jp in range(p2 * 4, p2 * 4 + 4):
                            out_proj(jp)
                return tail

            if F_XBLK:
                prev_tail = None
                for h in range(F_HLIM):
                    for p2 in range(2):
                        qt, qo = h // 2, (h % 2) * 64
                        kt = 4 + h // 2
                        pso = psop.tile([P, 2, 512], FP32, tag="pso")

                        def attnv(pex, pj, stop, pso=pso, h=h):
                            for q in range(2):
                                nc.tensor.matmul(
                                    pso[0:65, q, :],
                                    v_aug[:, pj, h * 65 : (h + 1) * 65],
                                    pex[:, q, :],
                                    start=(pj == 0),
                                    stop=stop,
                                )

                        pending = []
                        for j in range(JT):
                            pss = pssp.tile([P, 2, 512], FP32, tag="mm")
                            for q in range(2):
                                ib = p2 * 2 + q
                                nc.tensor.matmul(
                                    pss[:, q, :],
                                    qkT[qo : qo + 64, kt, j * P : (j + 1) * P],
                                    qkT[qo : qo + 64, qt, ib * 512 : (ib + 1) * 512],
                                    start=True,
                                    stop=True,
                                )
                            ex = expp.tile([P, 2, 512], MM_DT, tag="ex")
                            nc.scalar.activation(
                                out=ex.rearrange("p a b -> p (a b)"),
                                in_=pss.rearrange("p a b -> p (a b)"),
                                func=exp_f,
                                scale=EXP_SCALE,
                            )
                            pending.append((ex, j))
                            if j == 1 and prev_tail is not None:
                                # previous block's attnv tail runs here, after
                                # this block's first two sims, so the Act
                                # engine never starves at the block boundary
                                prev_tail()
                                prev_tail = None
                            if len(pending) > 2:
                                pex, pj = pending.pop(0)
                                attnv(pex, pj, stop=False)
                        prev_tail = make_tail(pending, attnv, pso, h, p2, qt, qo)
                if prev_tail is not None:
                    prev_tail()

            if F_P2OUT:
                hp_order = [(h, p2) for p2 in range(2) for h in range(F_HLIM)]
            elif F_XBLK:
                hp_order = []
            else:
                hp_order = [(h, p2) for h in range(F_HLIM) for p2 in range(2)]
            for h, p2 in hp_order:
                if True:
                    qt, qo = h // 2, (h % 2) * 64
                    kt = 4 + h // 2
                    pso = psop.tile([P, 2, 512], FP32)

                    def attnv(pex, pj, stop):
                        for q in range(2):
                            nc.tensor.matmul(
                                pso[0:65, q, :],
                                v_aug[:, pj, h * 65 : (h + 1) * 65],
                                pex[:, q, :],
                                start=(pj == 0),
                                stop=stop,
                            )

                    lag = int(os.environ.get("F_LAG", "3")) if F_LAG2 else 1
                    pending = []
                    for j in range(JT):
                        pss = pssp.tile([P, 2, 512], FP32, tag="mm")
                        for q in range(2):
                            ib = p2 * 2 + q
                            nc.tensor.matmul(
                                pss[:, q, :],
                                qkT[qo : qo + 64, kt, j * P : (j + 1) * P],
                                qkT[qo : qo + 64, qt, ib * 512 : (ib + 1) * 512],
                                start=True,
                                stop=True,
                            )
                        ex = expp.tile([P, 2, 512], MM_DT, tag="ex")
                        if j in DVE_J:
                            # Schraudolph: bf16 bits of e^s are an affine
                            # function of the (pre-scaled) psum scores
                            with nc.allow_low_precision("exp bits"):
                                nc.vector.tensor_scalar_add(
                                    out=ex.rearrange("p a b -> p (a b)").bitcast(I16),
                                    in0=pss.rearrange("p a b -> p (a b)"),
                                    scalar1=SCH_BIAS,
                                )
                        else:
                            nc.scalar.activation(
                                out=ex.rearrange("p a b -> p (a b)"),
                                in_=pss.rearrange("p a b -> p (a b)"),
                                func=exp_f,
                                scale=EXP_SCALE,
                            )
                        pending.append((ex, j))
                        if len(pending) > lag:
                            pex, pj = pending.pop(0)
                            attnv(pex, pj, stop=False)
                    for pex, pj in pending:
                        attnv(pex, pj, stop=(pj == JT - 1))

                    if F_LAG2:
                        # evacuate PSUM early (DVE) so the single pso buffer
                        # frees for the next block; normalize from SBUF off
                        # the critical path
                        pso_sb = psosbp.tile([65, 2, 512], FP32, tag="ps")
                        if F_EVAC == "act":
                            nc.scalar.copy(
                                out=pso_sb.rearrange("p a b -> p (a b)"),
                                in_=pso[0:65, :, :].rearrange("p a b -> p (a b)"),
                            )
                        else:
                            nc.vector.tensor_copy(
                                out=pso_sb.rearrange("p a b -> p (a b)"),
                                in_=pso[0:65, :, :].rearrange("p a b -> p (a b)"),
                            )
                        src = pso_sb
                    else:
                        src = pso
                    if F_DIV:
                        rb = rinvp.tile([64, 2, 512], FP32, tag="rb")
                        nc.gpsimd.partition_broadcast(rb, src[64:65, :, :])
                        with nc.allow_low_precision("attn out"):
                            nc.gpsimd.tensor_tensor(
                                out=attn_outT[
                                    qo : qo + 64, qt, p2 * 1024 : (p2 + 1) * 1024
                                ].rearrange("p (a b) -> p a b", a=2),
                                in0=src[0:64, :, :],
                                in1=rb,
                                op=mybir.AluOpType.divide,
                            )
                        if F_P2OUT and h == F_HLIM - 1:
                            for jp in range(p2 * 4, p2 * 4 + 4):
                                out_proj(jp)
                        continue
                    rinv = rinvp.tile([1, 2, 512], FP32, tag="rinv")
                    nc.vector.reciprocal(out=rinv, in_=src[64:65, :, :])
                    att_dst = attn_outT[
                        qo : qo + 64, qt, p2 * 1024 : (p2 + 1) * 1024
                    ].rearrange("p (a b) -> p a b", a=2)
                    with nc.allow_low_precision("attn out"):
                        if F_POOLMUL:
                            # broadcast + multiply both on the (mostly idle)
                            # gpsimd, keeping the DVE free for exp work
                            rb = rinvp.tile([64, 2, 512], FP32, tag="rb")
                            nc.gpsimd.partition_broadcast(rb, rinv)
                            nc.gpsimd.tensor_mul(
                                out=att_dst, in0=src[0:64, :, :], in1=rb
                            )
                        else:
                            rb = rinvp.tile([64, 2, 512], FP32, tag="rb")
                            nc.gpsimd.partition_broadcast(rb, rinv)
                            nc.vector.tensor_mul(
                                out=att_dst, in0=src[0:64, :, :], in1=rb
                            )
                    if F_P2OUT and h == F_HLIM - 1:
                        for jp in range(p2 * 4, p2 * 4 + 4):
                            out_proj(jp)

            # ---- out = attn_outT.T @ w_out (remaining tiles) ----
            if not F_P2OUT and not F_XBLK:
                for jp in range(8):
                    out_proj(jp)


_CACHE: dict = {}


def build_nc(repeat: int = 1) -> "bass.Bass":
    key = ("nc", repeat, F_BF16, F_ACTCP, F_DVE, F_LAG2)
    if key not in _CACHE:
        nc = bacc.Bacc("TRN2", target_bir_lowering=False, debug=False)
        with tile.TileContext(nc) as tc:
            _attention_body(tc, repeat=repeat)
        nc.compile()
        _CACHE[key] = nc
    return _CACHE[key]


def kernel(x: np.ndarray, w_qkv: np.ndarray, w_out: np.ndarray) -> np.ndarray:
    from concourse.bass_utils import run_bass_kernel_spmd

    nc = build_nc()
    x = np.ascontiguousarray(np.asarray(x, dtype=np.float32))
    w_qkv = np.ascontiguousarray(np.asarray(w_qkv, dtype=np.float32))
    w_out = np.ascontiguousarray(np.asarray(w_out, dtype=np.float32))
    in_maps = [
        {"x": x[i], "w_qkv": w_qkv, "w_out": w_out} for i in range(B)
    ]
    res = run_bass_kernel_spmd(nc, in_maps, core_ids=list(range(B)))
    return np.stack([r["out"] for r in res.results], axis=0)


# revision 26
# speedup vs baseline: 1.1640x; 1.1640x over previous
"""Multi-head attention forward (b=8, n=2048, dim=512, heads=8, dh=64) on 8
Trainium2 NeuronCores.

Sharding: data-parallel over batch — core i computes the full attention layer
for batch element i (weights replicated, no collectives).

Per-core pipeline (everything "transposed" so softmax rowsums come out of the
same matmuls):
  1. x [2048,512] -> PE-transpose -> xT [512,2048]
  2. qkT = w_qk.T @ xT           [1024,2048]  (q/k features on partitions;
     the q half is pre-scaled at weight-staging time)
  3. v   = x @ w_v               [2048,512]   (tokens on partitions) + ones col
  4. per head h, per pair of 512-wide query blocks:
       simT[j,i]  = kT_h[:,j].T @ qT_h[:,i]      (K=64, bf16 operands)
       exp        = Act engine Exp -> bf16       (256 x [128,1024] ops)
       outT_aug   = sum_j v_aug[j].T @ exp       (row 64 = exp rowsum)
       rinv       = 1/rowsum (DVE); bcast + multiply on gpsimd
       attn_outT[hd, i] = outT_aug[0:64] * rinv_bcast
  5. out = attn_outT.T @ w_out   [2048,512]

Tuned on hardware (defaults reflect the fastest measured config):
  - qkT/v/exp/attn_out/w_out in bf16: halves PE stationary-load (LDWEIGHTS)
    bytes in the attention inner loop (-25% wall). xT/w_qkv stay f32r (the
    all-bf16 projection variant miscomputed and was slower anyway).
  - attnv lags the sim matmul by 3 iterations (pss bufs=3, pso bufs=1, early
    PSUM->SBUF evacuation) so each exp has a ~2.5us window off the PE's
    critical path (-15% wall vs lag-1).
  - every PSUM->SBUF copy lives on the Act engine; the normalize multiply
    lives on gpsimd. The DVE is kept nearly empty: on this silicon its
    ops cost ~2x the cost model (mandatory inter-op DRAIN), and every
    attempt to offload exp (Schraudolph bit-trick) or copies onto it
    measured slower on HW despite the model predicting wins.

Env knobs (F_*) remain for hardware bisection; defaults are the tuned config.
"""

import os

import numpy as np

import concourse.bass as bass
import concourse.mybir as mybir
import concourse.tile as tile
from concourse import bacc
from concourse.masks import make_identity

FP32 = mybir.dt.float32
F32R = mybir.dt.float32r
BF16 = mybir.dt.bfloat16
I16 = mybir.dt.int16

B = 8
N = 2048
D = 512
H = 8
DH = 64
F3 = 3 * D
SCALE = DH**-0.5
P = 128
NT = N // P  # 16 token tiles
CT = D // P  # 4 contraction tiles over dim
JT = N // P  # 16 key tiles

# Schraudolph exp-in-bf16-bits: for w = e^s, bf16 bits ~= A16*s + 16256 - C
A16 = 184.6649652337873  # 128 * log2(e)
C_SCH = 5.5  # minimax-tuned bit offset
SCH_BIAS = 16256.0 - C_SCH + 0.5  # +0.5 so a truncating f32->i16 cast rounds
INV_A16 = 1.0 / A16

F_BF16 = os.environ.get("F_BF16", "1") == "1"
F_ACTCP = os.environ.get("F_ACTCP", "1") == "1"
F_LAG2 = os.environ.get("F_LAG2", "1") == "1"
# number of j-tiles (of 16) whose exp runs on the DVE (Schraudolph bits)
F_NDVE = int(os.environ.get("F_NDVE", "0"))
# normalize: 1 = recip on DVE + multiply on gpsimd (broadcast AP), 0 = old
# DVE recip + gpsimd partition_broadcast + DVE multiply
F_POOLMUL = os.environ.get("F_POOLMUL", "1") == "1"
# v_aug PSUM->SBUF copies: 1 = Act engine, 0 = DVE
F_VACT = os.environ.get("F_VACT", "1") == "1"
# pso evacuation engine: "act" or "dve"
F_EVAC = os.environ.get("F_EVAC", "act")
# timing-only knob: process only the first F_HLIM heads (wrong numerics)
F_HLIM = int(os.environ.get("F_HLIM", str(H)))
# sub-knobs: bf16 for qkT (sim operands) / xT+wqkv (proj operands)
F_QKBF = os.environ.get("F_QKBF", "1") == "1"
F_XWBF = os.environ.get("F_XWBF", "0") == "1"
# normalize divide on gpsimd (no DVE recip): 1 = bcast rowsum + divide
F_DIV = os.environ.get("F_DIV", "0") == "1"
# p2-outer loop order with early out-projection per half
F_P2OUT = os.environ.get("F_P2OUT", "0") == "1"
# cross-block software pipeline: defer each block's attnv tail until after
# the next block's first two sim matmuls (needs pss bufs=2 / pso bufs=2)
F_XBLK = os.environ.get("F_XBLK", "0") == "1"
# fuse the per-j pair of 512-wide sim/attnv matmuls into one 1024-wide matmul
F_FUSE = os.environ.get("F_FUSE", "0") == "1"

F_DVE = F_NDVE > 0
MM_DT = BF16 if F_BF16 else F32R
QSCALE = (A16 * SCALE) if F_DVE else SCALE
EXP_SCALE = INV_A16 if F_DVE else 1.0
# j-tiles whose exp runs on the DVE instead of the Act engine, spread out
DVE_J = frozenset(
    (i * JT // F_NDVE + JT // (2 * F_NDVE)) for i in range(F_NDVE)
) if F_DVE else frozenset()


def _attention_body(tc: "tile.TileContext", repeat: int = 1):
    nc = tc.nc
    x = nc.dram_tensor("x", [N, D], FP32, kind="ExternalInput").ap()
    w_qkv = nc.dram_tensor("w_qkv", [D, F3], FP32, kind="ExternalInput").ap()
    w_out = nc.dram_tensor("w_out", [D, D], FP32, kind="ExternalInput").ap()
    out = nc.dram_tensor("out", [N, D], FP32, kind="ExternalOutput").ap()
    for _ in range(repeat):
        _attention_once(tc, x, w_qkv, w_out, out)


def _attention_once(tc: "tile.TileContext", x, w_qkv, w_out, out):
    nc = tc.nc
    exp_f = mybir.ActivationFunctionType.Exp

    with (
        tc.tile_pool(name="const", bufs=1) as const,
        tc.tile_pool(name="persist", bufs=1) as persist,
        tc.tile_pool(name="wstage", bufs=1) as wstage,
    ):
        identity = const.tile([P, P], FP32)
        make_identity(nc, identity)
        ones32 = const.tile([P, 1], FP32)
        nc.vector.memset(ones32, 1.0)

        # w_out: DMA fp32 -> rounding copy
        wout_sb = persist.tile([P, CT, D], MM_DT)
        with nc.allow_low_precision("weights"):
            for t in range(CT):
                ws = wstage.tile([P, F3], FP32, tag="ws")
                nc.sync.dma_start(out=ws[:, :D], in_=w_out[t * P : (t + 1) * P, :])
                nc.vector.tensor_copy(out=wout_sb[:, t, :], in_=ws[:, :D])

        # q and k features transposed: rows = 1024 q/k features in 8 tiles
        qkT = persist.tile([P, 8, N], MM_DT if F_QKBF else F32R)
        # v with tokens on partitions; per head 64 value cols + 1 ones col
        v_aug = persist.tile([P, JT, H * 65], MM_DT)
        with nc.allow_low_precision("v"):
            nc.vector.tensor_copy(
                out=v_aug.rearrange("p j (h c) -> p j h c", c=65)[:, :, :, 64:65],
                in_=ones32.to_broadcast([P, JT, H, 1]),
            )

        with (
            tc.tile_pool(name="proj", bufs=1) as proj_pool,
            tc.tile_pool(name="xstage", bufs=3) as xstage,
            tc.tile_pool(name="pst", bufs=3, space="PSUM") as pst,
            tc.tile_pool(name="psmm", bufs=2, space="PSUM") as psmm,
        ):
            # w_qkv staging; the q columns (0:512) are pre-scaled
            wqkv_sb = proj_pool.tile([P, CT, F3], MM_DT if F_XWBF else F32R)
            for t in range(CT):
                ws = wstage.tile([P, F3], FP32, tag="ws")
                nc.sync.dma_start(out=ws, in_=w_qkv[t * P : (t + 1) * P, :])
                nc.vector.tensor_scalar_mul(
                    out=wqkv_sb[:, t, 0:D], in0=ws[:, 0:D], scalar1=QSCALE
                )
                nc.vector.tensor_copy(out=wqkv_sb[:, t, D:F3], in_=ws[:, D:F3])

            # ---- load x and transpose to xT [512, 2048] ----
            xT = proj_pool.tile([P, CT, N], MM_DT if F_XWBF else F32R)
            for j in range(NT):
                xs = xstage.tile([P, D], FP32)
                nc.sync.dma_start(out=xs, in_=x[j * P : (j + 1) * P, :])
                ps = pst.tile([P, CT, P], FP32)  # one bank, 4 transposes
                for t in range(CT):
                    nc.tensor.transpose(
                        ps[:, t, :], xs[:, t * P : (t + 1) * P], identity
                    )
                nc.vector.tensor_copy(out=xT[:, :, j * P : (j + 1) * P], in_=ps)

            # ---- v = x @ w_v (tokens on partitions) ----
            for jp in range(8):
                ps = psmm.tile([P, 2, 512], FP32, tag="mm")
                for c in range(CT):
                    for q in range(2):
                        j = jp * 2 + q
                        nc.tensor.matmul(
                            ps[:, q, :],
                            xT[:, c, j * P : (j + 1) * P],
                            wqkv_sb[:, c, 2 * D : 3 * D],
                            start=(c == 0),
                            stop=(c == CT - 1),
                        )
                with nc.allow_low_precision("v"):
                    v_dst = v_aug[:, jp * 2 : jp * 2 + 2, :].rearrange(
                        "p j (h c) -> p j h c", c=65
                    )[:, :, :, 0:64]
                    v_src = ps.rearrange("p q (h c) -> p q h c", c=64)
                    if F_VACT:
                        nc.scalar.copy(out=v_dst, in_=v_src)
                    else:
                        nc.vector.tensor_copy(out=v_dst, in_=v_src)

            # ---- qkT = w_qk.T @ xT ----
            # m-order pairs each head's q tile with its k tile so head 0's
            # attention can start as early as possible
            for m in (0, 4, 1, 5, 2, 6, 3, 7):
                for nbp in range(2):
                    ps = psmm.tile([P, 2, 512], FP32, tag="mm")
                    for c in range(CT):
                        for q in range(2):
                            nc.tensor.matmul(
                                ps[:, q, :],
                                wqkv_sb[:, c, m * P : (m + 1) * P],
                                xT[:, c, (nbp * 2 + q) * 512 : (nbp * 2 + q + 1) * 512],
                                start=(c == 0),
                                stop=(c == CT - 1),
                            )
                    if F_ACTCP:
                        nc.scalar.copy(
                            out=qkT[:, m, nbp * 1024 : (nbp + 1) * 1024],
                            in_=ps.rearrange("p a b -> p (a b)"),
                        )
                    else:
                        nc.vector.tensor_copy(
                            out=qkT[:, m, nbp * 1024 : (nbp + 1) * 1024],
                            in_=ps.rearrange("p a b -> p (a b)"),
                        )

        # ---- attention per head, i-blocks in pairs ----
        with (
            tc.tile_pool(name="attno", bufs=1) as attno_pool,
            tc.tile_pool(name="expp", bufs=int(os.environ.get("F_EXPB", "5")) if F_LAG2 else 3) as expp,
            tc.tile_pool(name="rinvp", bufs=2) as rinvp,
            tc.tile_pool(name="psosb", bufs=2) as psosbp,
            tc.tile_pool(name="outstage", bufs=2) as outstage,
            tc.tile_pool(name="pss", bufs=3 if F_LAG2 else 2, space="PSUM") as pssp,
            tc.tile_pool(name="pso", bufs=1 if F_LAG2 else 2, space="PSUM") as psop,
        ):
            attn_outT = attno_pool.tile([P, CT, N], MM_DT)
            if F_HLIM < H:
                with nc.allow_low_precision("timing stub"):
                    nc.vector.memset(attn_outT, 0.25)

            def out_proj(jp):
                ps = pssp.tile([P, 2, 512], FP32, tag="mm")
                for t in range(CT):
                    for q in range(2):
                        j = jp * 2 + q
                        nc.tensor.matmul(
                            ps[:, q, :],
                            attn_outT[:, t, j * P : (j + 1) * P],
                            wout_sb[:, t, :],
                            start=(t == 0),
                            stop=(t == CT - 1),
                        )
                os_ = outstage.tile([P, 2, D], FP32)
                if F_ACTCP:
                    nc.scalar.copy(
                        out=os_.rearrange("p a b -> p (a b)"),
                        in_=ps.rearrange("p a b -> p (a b)"),
                    )
                else:
                    nc.vector.tensor_copy(out=os_, in_=ps)
                nc.sync.dma_start(
                    out=out[jp * 256 : (jp + 1) * 256, :].rearrange(
                        "(q p) d -> p q d", p=P
                    ),
                    in_=os_,
                )

            def make_tail(pending, attnv, pso, h, p2, qt, qo):
                # flush the last two attnv matmuls, evacuate PSUM, normalize,
                # and (after the final head's half) start that half's
                # out-projection
                def tail():
                    for pex, pj in pending:
                        attnv(pex, pj, stop=(pj == JT - 1))
                    pso_sb = psosbp.tile([65, 2, 512], FP32, tag="ps")
                    nc.scalar.copy(
                        out=pso_sb.rearrange("p a b -> p (a b)"),
                        in_=pso[0:65, :, :].rearrange("p a b -> p (a b)"),
                    )
                    rinv = rinvp.tile([1, 2, 512], FP32, tag="rinv")
                    nc.vector.reciprocal(out=rinv, in_=pso_sb[64:65, :, :])
                    rb = rinvp.tile([64, 2, 512], FP32, tag="rb")
                    nc.gpsimd.partition_broadcast(rb, rinv)
                    with nc.allow_low_precision("attn out"):
                        nc.gpsimd.tensor_mul(
                            out=attn_outT[
                                qo : qo + 64, qt, p2 * 1024 : (p2 + 1) * 1024
                            ].rearrange("p (a b) -> p a b", a=2),
                            in0=pso_sb[0:64, :, :],
                            in1=rb,
                        )
                    if h == F_HLIM - 1:
                        for jp in range(p2 * 4, p2 * 4 + 4):
                            out_proj(jp)
                return tail

            if F_XBLK:
                prev_tail = None
                for h in range(F_HLIM):
                    for p2 in range(2):
                        qt, qo = h // 2, (h % 2) * 64
                        kt = 4 + h // 2
                        pso = psop.tile([P, 2, 512], FP32, tag="pso")

                        def attnv(pex, pj, stop, pso=pso, h=h):
                            for q in range(2):
                                nc.tensor.matmul(
                                    pso[0:65, q, :],
                                    v_aug[:, pj, h * 65 : (h + 1) * 65],
                                    pex[:, q, :],
                                    start=(pj == 0),
                                    stop=stop,
                                )

                        pending = []
                        for j in range(JT):
                            pss = pssp.tile([P, 2, 512], FP32, tag="mm")
                            for q in range(2):
                                ib = p2 * 2 + q
                                nc.tensor.matmul(
                                    pss[:, q, :],
                                    qkT[qo : qo + 64, kt, j * P : (j + 1) * P],
                                    qkT[qo : qo + 64, qt, ib * 512 : (ib + 1) * 512],
                                    start=True,
                                    stop=True,
                                )
                            ex = expp.tile([P, 2, 512], MM_DT, tag="ex")
                            nc.scalar.activation(
                                out=ex.rearrange("p a b -> p (a b)"),
                                in_=pss.rearrange("p a b -> p (a b)"),
                                func=exp_f,
                                scale=EXP_SCALE,
                            )
                            pending.append((ex, j))
                            if j == 1 and prev_tail is not None:
                                # previous block's attnv tail runs here, after
                                # this block's first two sims, so the Act
                                # engine never starves at the block boundary
                                prev_tail()
                                prev_tail = None
                            if len(pending) > 2:
                                pex, pj = pending.pop(0)
                                attnv(pex, pj, stop=False)
                        prev_tail = make_tail(pending, attnv, pso, h, p2, qt, qo)
                if prev_tail is not None:
                    prev_tail()

            if F_P2OUT:
                hp_order = [(h, p2) for p2 in range(2) for h in range(F_HLIM)]
            elif F_XBLK:
                hp_order = []
            else:
                hp_order = [(h, p2) for h in range(F_HLIM) for p2 in range(2)]
            for h, p2 in hp_order:
                if True:
                    qt, qo = h // 2, (h % 2) * 64
                    kt = 4 + h // 2
                    pso = psop.tile([P, 2, 512], FP32)

                    def attnv(pex, pj, stop):
                        if F_FUSE:
                            nc.tensor.matmul(
                                pso[0:65, :, :].rearrange("p a b -> p (a b)"),
                                v_aug[:, pj, h * 65 : (h + 1) * 65],
                                pex.rearrange("p a b -> p (a b)"),
                                start=(pj == 0),
                                stop=stop,
                            )
                        else:
                            for q in range(2):
                                nc.tensor.matmul(
                                    pso[0:65, q, :],
                                    v_aug[:, pj, h * 65 : (h + 1) * 65],
                                    pex[:, q, :],
                                    start=(pj == 0),
                                    stop=stop,
                                )

                    lag = int(os.environ.get("F_LAG", "3")) if F_LAG2 else 1
                    pending = []
                    for j in range(JT):
                        pss = pssp.tile([P, 2, 512], FP32, tag="mm")
                        if F_FUSE:
                            nc.tensor.matmul(
                                pss.rearrange("p a b -> p (a b)"),
                                qkT[qo : qo + 64, kt, j * P : (j + 1) * P],
                                qkT[qo : qo + 64, qt, p2 * 1024 : (p2 + 1) * 1024],
                                start=True,
                                stop=True,
                            )
                        else:
                            for q in range(2):
                                ib = p2 * 2 + q
                                nc.tensor.matmul(
                                    pss[:, q, :],
                                    qkT[qo : qo + 64, kt, j * P : (j + 1) * P],
                                    qkT[qo : qo + 64, qt, ib * 512 : (ib + 1) * 512],
                                    start=True,
                                    stop=True,
                                )
                        ex = expp.tile([P, 2, 512], MM_DT, tag="ex")
                        if j in DVE_J:
                            # Schraudolph: bf16 bits of e^s are an affine
                            # function of the (pre-scaled) psum scores
                            with nc.allow_low_precision("exp bits"):
                                nc.vector.tensor_scalar_add(
                                    out=ex.rearrange("p a b -> p (a b)").bitcast(I16),
                                    in0=pss.rearrange("p a b -> p (a b)"),
                                    scalar1=SCH_BIAS,
                                )
                        else:
                            nc.scalar.activation(
                                out=ex.rearrange("p a b -> p (a b)"),
                                in_=pss.rearrange("p a b -> p (a b)"),
                                func=exp_f,
                                scale=EXP_SCALE,
                            )
                        pending.append((ex, j))
                        if len(pending) > lag:
                            pex, pj = pending.pop(0)
                            attnv(pex, pj, stop=False)
                    for pex, pj in pending:
                        attnv(pex, pj, stop=(pj == JT - 1))

                    if F_LAG2:
                        # evacuate PSUM early (DVE) so the single pso buffer
                        # frees for the next block; normalize from SBUF off
                        # the critical path
                        pso_sb = psosbp.tile([65, 2, 512], FP32, tag="ps")
                        if F_EVAC == "act":
                            nc.scalar.copy(
                                out=pso_sb.rearrange("p a b -> p (a b)"),
                                in_=pso[0:65, :, :].rearrange("p a b -> p (a b)"),
                            )
                        else:
                            nc.vector.tensor_copy(
                                out=pso_sb.rearrange("p a b -> p (a b)"),
                                in_=pso[0:65, :, :].rearrange("p a b -> p (a b)"),
                            )
                        src = pso_sb
                    else:
                        src = pso
                    if F_DIV:
                        rb = rinvp.tile([64, 2, 512], FP32, tag="rb")
                        nc.gpsimd.partition_broadcast(rb, src[64:65, :, :])
                        with nc.allow_low_precision("attn out"):
                            nc.gpsimd.tensor_tensor(
                                out=attn_outT[
                                    qo : qo + 64, qt, p2 * 1024 : (p2 + 1) * 1024
                                ].rearrange("p (a b) -> p a b", a=2),
                                in0=src[0:64, :, :],
                                in1=rb,
                                op=mybir.AluOpType.divide,
                            )
                        if F_P2OUT and h == F_HLIM - 1:
                            for jp in range(p2 * 4, p2 * 4 + 4):
                                out_proj(jp)
                        continue
                    rinv = rinvp.tile([1, 2, 512], FP32, tag="rinv")
                    nc.vector.reciprocal(out=rinv, in_=src[64:65, :, :])
                    att_dst = attn_outT[
                        qo : qo + 64, qt, p2 * 1024 : (p2 + 1) * 1024
                    ].rearrange("p (a b) -> p a b", a=2)
                    with nc.allow_low_precision("attn out"):
                        if F_POOLMUL:
                            # broadcast + multiply both on the (mostly idle)
                            # gpsimd, keeping the DVE free for exp work
                            rb = rinvp.tile([64, 2, 512], FP32, tag="rb")
                            nc.gpsimd.partition_broadcast(rb, rinv)
                            nc.gpsimd.tensor_mul(
                                out=att_dst, in0=src[0:64, :, :], in1=rb
                            )
                        else:
                            rb = rinvp.tile([64, 2, 512], FP32, tag="rb")
                            nc.gpsimd.partition_broadcast(rb, rinv)
                            nc.vector.tensor_mul(
                                out=att_dst, in0=src[0:64, :, :], in1=rb
                            )
                    if F_P2OUT and h == F_HLIM - 1:
                        for jp in range(p2 * 4, p2 * 4 + 4):
                            out_proj(jp)

            # ---- out = attn_outT.T @ w_out (remaining tiles) ----
            if not F_P2OUT and not F_XBLK:
                for jp in range(8):
                    out_proj(jp)


_CACHE: dict = {}


def build_nc(repeat: int = 1) -> "bass.Bass":
    key = ("nc", repeat, F_BF16, F_ACTCP, F_DVE, F_LAG2)
    if key not in _CACHE:
        nc = bacc.Bacc("TRN2", target_bir_lowering=False, debug=False)
        with tile.TileContext(nc) as tc:
            _attention_body(tc, repeat=repeat)
        nc.compile()
        _CACHE[key] = nc
    return _CACHE[key]


def kernel(x: np.ndarray, w_qkv: np.ndarray, w_out: np.ndarray) -> np.ndarray:
    from concourse.bass_utils import run_bass_kernel_spmd

    nc = build_nc()
    x = np.ascontiguousarray(np.asarray(x, dtype=np.float32))
    w_qkv = np.ascontiguousarray(np.asarray(w_qkv, dtype=np.float32))
    w_out = np.ascontiguousarray(np.asarray(w_out, dtype=np.float32))
    in_maps = [
        {"x": x[i], "w_qkv": w_qkv, "w_out": w_out} for i in range(B)
    ]
    res = run_bass_kernel_spmd(nc, in_maps, core_ids=list(range(B)))
    return np.stack([r["out"] for r in res.results], axis=0)
